# revision 16
# baseline (speedup 1.0000x reference)
"""Trainium2 Bass kernel for the CGF tree-GRU problem (v2).

Problem: 3-level complete 8-ary tree GRU (torch GRU cell convention).
  Level 3: 64 nodes x 8 embedded leaf children, h0 = 0
  Level 2:  8 nodes x 8 children (level-3 outputs), h0 = mean of children h
  Level 1:  1 node  x 8 children (level-2 outputs), h0 = mean of children h
  Output: mean over the 8 step outputs of the root GRU. D = 512.

One serial chain of 24 GRU steps, replicated SPMD on all 8 cores (a
per-step collective costs more than a step: AllGather floor ~4.6us).

v2 changes vs the DoubleRow baseline (154us):
  - Recurrent matmuls: fp8e3 (e3m4, 4 mantissa bits) weights WITHOUT
    DoubleRow -> Fast Weight Load.  DR at free-dim<=64 costs ~127ns/pair
    vs ~40-53ns for FWL pairs.  rhs h stays bf16 (mixed-dtype matmul).
  - Single bf16 h state (no fp8 state copy), PSUM domain = 64x true.
  - Per-step gate-input adds eliminated: gi is injected into PSUM by
    identity-weight matmuls inside each burst; sigma reads PSUM directly.
  - n-gate hidden bias injected via a ones-row matmul (k=1 tile).
  - Tail shortened to w = n - h; ft = z*w; h' = n - ft (3 DVE bf16 ops).
  - ACT order per step: sigmoid(r) -> tanh -> sigmoid(z).
  - PE warmed with dummy matmuls during the input DMA so HAM is at
    2.4GHz when the real work starts.
"""

import numpy as np

import concourse.bacc as bacc
import concourse.mybir as mybir
from concourse.tile import TileContext
from concourse.bass_utils import run_bass_kernel_spmd

AF = mybir.ActivationFunctionType
OP = mybir.AluOpType
PM = mybir.MatmulPerfMode.DoubleRow
FP = mybir.dt.float32
BF = mybir.dt.bfloat16
F8E4 = mybir.dt.float8e4
F8E3 = mybir.dt.float8e3

P = 128          # partitions
D = 512          # hidden size
KT = D // P      # 4 k-tiles
MT = 12          # gate m-tiles (3*512/128)
A = 8            # tree arity == sequence length per level
NB = 64          # level-3 node count
T = 8            # steps per level
N_CORES = 8
WS = 64.0        # weight pre-scale -> PSUM domain is 64x true values
XS = 16.0        # level-3 embedding pre-scale (fp8e4 subnormal lift)
TNB = T * NB     # 512 level-3 sequence columns
N_WARM = 22      # PE warm-up matmuls during input DMA

# blob_a (fp8e4): [xt (KT*TNB) | wit8 (MT*KT*P)]
OA_XT = 0
OA_WIT8 = OA_XT + KT * TNB
A_COLS = OA_WIT8 + MT * KT * P
# blob_b (bf16): [wrec (MT*KT*P) | wit3 (MT*KT*P)]
OB_WREC = 0
OB_WIT3 = OB_WREC + MT * KT * P
B_COLS = OB_WIT3 + MT * KT * P
# blob_c (bf16): [identity (P) | bhn_row (D), partition 0 only]
OC_ID = 0
OC_BHN = OC_ID + P
C_COLS = OC_BHN + D
# blob_d (fp32): [gb64 (MT) | bhnb64 (KT*NB)]
OD_GB = 0
OD_BHNB = OD_GB + MT
D_COLS = OD_BHNB + KT * NB

_BUILT = None


def _build_nc():
    nc = bacc.Bacc()

    blob_a = nc.declare_dram_parameter("blob_a", [P, A_COLS], F8E4, isOutput=False)
    blob_b = nc.declare_dram_parameter("blob_b", [P, B_COLS], BF, isOutput=False)
    blob_c = nc.declare_dram_parameter("blob_c", [P, C_COLS], BF, isOutput=False)
    blob_d = nc.declare_dram_parameter("blob_d", [P, D_COLS], FP, isOutput=False)
    outp = nc.declare_dram_parameter("out", [P, KT], FP, isOutput=True)

    with TileContext(nc) as tc:
        with (
            tc.tile_pool(name="const", bufs=1) as cpool,
            tc.tile_pool(name="state", bufs=1) as spool,
            tc.tile_pool(name="work", bufs=2) as wpool,
            tc.tile_pool(name="pg", bufs=2, space="PSUM") as gpool,
            tc.tile_pool(name="pw", bufs=1, space="PSUM") as wmpool,
            tc.tile_pool(name="pr", bufs=1, space="PSUM") as prpool,
            tc.tile_pool(name="pz", bufs=1, space="PSUM") as pzpool,
            tc.tile_pool(name="pn", bufs=1, space="PSUM") as pnpool,
        ):
            # ---------------- input DMA (consumption order) ----------------
            # Two queues (sync + scalar), large chunks, issued before the ACT
            # table loads so transfers start immediately.
            c_sb = cpool.tile([P, C_COLS], BF)
            nc.scalar.dma_start(out=c_sb[:], in_=blob_c[:, :])
            d_sb = cpool.tile([P, D_COLS], FP)
            nc.scalar.dma_start(out=d_sb[:], in_=blob_d[:, :])
            a_sb = cpool.tile([P, A_COLS], F8E4)
            nc.sync.dma_start(out=a_sb[:, 0:4096], in_=blob_a[:, 0:4096])
            nc.scalar.dma_start(out=a_sb[:, 4096:A_COLS], in_=blob_a[:, 4096:A_COLS])
            b_sb = cpool.tile([P, B_COLS], BF)
            nc.sync.dma_start(out=b_sb[:, 0:4096], in_=blob_b[:, 0:4096])
            nc.scalar.dma_start(out=b_sb[:, 4096:8192], in_=blob_b[:, 4096:8192])
            nc.sync.dma_start(out=b_sb[:, 8192:B_COLS], in_=blob_b[:, 8192:B_COLS])

            # ---------------- warm-ups ----------------
            # ACT spline tables: Sigmoid first so one ACT_TABLE_LOAD covers
            # Sigmoid/Tanh/Identity (the sigmoid set contains all three).
            wact = cpool.tile([P, 8], FP)
            nc.vector.memset(wact[:, :], 0.0)
            for fn in (AF.Sigmoid, AF.Tanh, AF.Identity):
                nc.scalar.activation(wact[:, :], wact[:, :], fn)
            # PE HAM: dummy matmuls on a zero tile while the DMA runs so the
            # clock gate is at 8/8 when the real matmuls arrive.
            warm = cpool.tile([P, P], BF)
            nc.vector.memset(warm[:, :], 0.0)
            warm_ps = wmpool.tile([P, P], FP)
            for _ in range(N_WARM):
                nc.tensor.matmul(warm_ps[:, :], lhsT=warm[:, :], rhs=warm[:, :],
                                 start=True, stop=True)

            ident = c_sb[:, OC_ID:OC_ID + P]
            gb_sb = d_sb[:, OD_GB:OD_GB + MT]
            bhnb = d_sb[:, OD_BHNB:OD_BHNB + KT * NB]
            bhnbv = bhnb.rearrange("p (k b) -> p k b", k=KT)
            xt8 = a_sb[:, OA_XT:OA_XT + KT * TNB]
            wit8 = a_sb[:, OA_WIT8:OA_WIT8 + MT * KT * P]
            wrec = b_sb[:, OB_WREC:OB_WREC + MT * KT * P]
            wit3 = b_sb[:, OB_WIT3:OB_WIT3 + MT * KT * P]

            def w8_tile(m, kk):
                # [P, 2, 128] DoubleRow stationary pair (k-tiles 2kk, 2kk+1)
                return wit8[
                    :, (m * KT + 2 * kk) * P:(m * KT + 2 * kk + 2) * P
                ].rearrange("p (two f) -> p two f", two=2)

            def wr_tile(m, k):
                return wrec[:, (m * KT + k) * P:(m * KT + k + 1) * P]

            def wi3_tile(m, k):
                return wit3[:, (m * KT + k) * P:(m * KT + k + 1) * P]

            def bhn_row(m):
                return c_sb[0:1, OC_BHN + m * P:OC_BHN + (m + 1) * P]

            # bf16 broadcast copy of the (64x) n-gate hidden bias, used as
            # the moving operand of the ps_n identity-inject matmul.
            bhnb_bf = cpool.tile([P, KT * NB], BF)
            nc.vector.tensor_copy(bhnb_bf[:, :], bhnb[:, :])
            bhnb_bfv = bhnb_bf.rearrange("p (k b) -> p k b", k=KT)

            def gi_matmul_fp8dr(gi, x_v):
                """Level-3 input matmul: fp8e4 DoubleRow, 256-col chunks."""
                for m in range(MT):
                    psb = gpool.tile([P, 512], FP, tag="gi", name="psb")
                    ps = psb[:, :TNB]
                    for c in range(2):
                        sl = ps[:, c * 256:(c + 1) * 256]
                        for kk in range(2):
                            nc.tensor.matmul(
                                sl,
                                lhsT=w8_tile(m, kk),
                                rhs=x_v[:, 2 * kk:2 * kk + 2, c * 256:(c + 1) * 256],
                                start=(kk == 0),
                                stop=(kk == 1),
                                perf_mode=PM,
                            )
                    dst = gi[:, m * TNB:(m + 1) * TNB]
                    if m % 2 == 0:
                        nc.vector.tensor_scalar_add(dst, ps[:, :], gb_sb[:, m:m + 1])
                    else:
                        nc.scalar.activation(dst, ps[:, :], AF.Identity,
                                             bias=gb_sb[:, m:m + 1], scale=1.0)

            def gi_matmul_e3(gi, x_v, cols):
                """Level-2/1 input matmul: bf16 weights x bf16 x (FWL)."""
                for m in range(MT):
                    psb = gpool.tile([P, 512], FP, tag="gi", name="psb")
                    ps = psb[:, :cols]
                    for k in range(KT):
                        nc.tensor.matmul(
                            ps,
                            lhsT=wi3_tile(m, k),
                            rhs=x_v[:, k],
                            start=(k == 0),
                            stop=(k == KT - 1),
                        )
                    dst = gi[:, m * cols:(m + 1) * cols]
                    if m % 2 == 0:
                        nc.vector.tensor_scalar_add(dst, ps[:, :], gb_sb[:, m:m + 1])
                    else:
                        nc.scalar.activation(dst, ps[:, :], AF.Identity,
                                             bias=gb_sb[:, m:m + 1], scale=1.0)

            def gru_level(B, h, acc, gi_v, zero_h0):
                """8 GRU steps.  h: [P, KT*B] bf16 (true scale); acc: [P, KT*B]
                fp32 output-mean accumulator; gi_v: [p, m, t, b] bf16 AP of the
                64x-domain biased input gates.

                Injects for step t+1 (gi via identity weights, bhn via a
                ones-row) are emitted at the end of step t so they run in the
                PE stream while step t's gate chain executes; the recurrent
                matmuls of t+1 then only wait on h."""
                W4 = 4 * B

                def v4(ap):
                    return ap.rearrange("p (m b) -> p m b", m=4)

                def emit_injects(t):
                    ps_r = prpool.tile([P, 512], FP, tag="ps_r", name="ps_r")
                    ps_z = pzpool.tile([P, 512], FP, tag="ps_z", name="ps_z")
                    ps_n = pnpool.tile([P, 512], FP, tag="ps_n", name="ps_n")
                    nc.tensor.matmul(v4(ps_r[:, :W4]), lhsT=ident,
                                     rhs=gi_v[:, 0:4, t], start=True, stop=False,
                                     skip_group_check=True)
                    nc.tensor.matmul(v4(ps_n[:, :W4]), lhsT=ident,
                                     rhs=bhnb_bfv[:, :, :B], start=True, stop=False,
                                     skip_group_check=True)
                    nc.tensor.matmul(v4(ps_z[:, :W4]), lhsT=ident,
                                     rhs=gi_v[:, 4:8, t], start=True, stop=False,
                                     skip_group_check=True)
                    return ps_r, ps_z, ps_n

                ps = None
                for t in range(T):
                    rt = wpool.tile([P, W4], BF, tag="rt")
                    zt = wpool.tile([P, W4], BF, tag="zt")
                    ctm = wpool.tile([P, W4], BF, tag="ctm")
                    ct = wpool.tile([P, W4], BF, tag="ct")
                    nt = wpool.tile([P, W4], BF, tag="nt")
                    dd = wpool.tile([P, W4], BF, tag="dd")
                    cc = wpool.tile([P, W4], BF, tag="cc")
                    ee = wpool.tile([P, W4], BF, tag="ee")

                    if t == 0 and zero_h0:
                        # h = 0: gates come straight from gi; h' = (1-z)*n
                        nc.scalar.activation(v4(rt[:, :]), gi_v[:, 0:4, 0],
                                             AF.Sigmoid, scale=1.0 / WS)
                        nc.scalar.activation(v4(zt[:, :]), gi_v[:, 4:8, 0],
                                             AF.Sigmoid, scale=1.0 / WS)
                        nc.vector.tensor_mul(v4(ctm[:, :]), v4(rt[:, :]),
                                             bhnbv[:, :, :B])
                        nc.vector.tensor_add(v4(ct[:, :]), v4(ctm[:, :]),
                                             gi_v[:, 8:12, 0])
                        nc.scalar.activation(nt[:, :], ct[:, :], AF.Tanh,
                                             scale=1.0 / WS)
                        nc.vector.tensor_scalar(dd[:, :], zt[:, :], -1.0, 1.0,
                                                OP.mult, OP.add)
                        nc.vector.tensor_mul(h[:, :], nt[:, :], dd[:, :])
                        nc.gpsimd.tensor_copy(acc[:, :], h[:, :])
                        ps = emit_injects(1)
                        continue

                    if ps is None:  # first step of a non-zero-h0 level
                        ps = emit_injects(t)
                    ps_r, ps_z, ps_n = ps

                    # recurrent bursts: r, n, z (gi/bhn already in PSUM)
                    for m in range(4):
                        sl = ps_r[:, m * B:(m + 1) * B]
                        for k in range(KT):
                            nc.tensor.matmul(sl, lhsT=wr_tile(m, k),
                                             rhs=h[:, k * B:(k + 1) * B],
                                             start=False,
                                             stop=(m == 3 and k == KT - 1),
                                             skip_group_check=True)
                    for m in range(4):
                        sl = ps_n[:, m * B:(m + 1) * B]
                        for k in range(KT):
                            nc.tensor.matmul(sl, lhsT=wr_tile(8 + m, k),
                                             rhs=h[:, k * B:(k + 1) * B],
                                             start=False,
                                             stop=(m == 3 and k == KT - 1),
                                             skip_group_check=True)
                    for m in range(4):
                        sl = ps_z[:, m * B:(m + 1) * B]
                        for k in range(KT):
                            nc.tensor.matmul(sl, lhsT=wr_tile(4 + m, k),
                                             rhs=h[:, k * B:(k + 1) * B],
                                             start=False,
                                             stop=(m == 3 and k == KT - 1),
                                             skip_group_check=True)

                    nc.scalar.activation(rt[:, :], ps_r[:, :W4], AF.Sigmoid,
                                         scale=1.0 / WS)
                    nc.scalar.activation(zt[:, :], ps_z[:, :W4], AF.Sigmoid,
                                         scale=1.0 / WS)

                    # ct path: ctm = rt * (ps_n + bhn); ct = ctm + gi_n
                    nc.vector.tensor_mul(v4(ctm[:, :]), v4(rt[:, :]),
                                         v4(ps_n[:, :W4]))
                    nc.vector.tensor_add(v4(ct[:, :]), v4(ctm[:, :]),
                                         gi_v[:, 8:12, t])
                    nc.scalar.activation(nt[:, :], ct[:, :], AF.Tanh,
                                         scale=1.0 / WS)

                    # next step's injects ride the PE stream here (all PSUM
                    # readers of step t are already emitted above).
                    ps = emit_injects(t + 1) if t + 1 < T else None

                    # tail: d = 1-z ; c = z*h ; e = n*d ; h' = e + c
                    nc.vector.tensor_scalar(dd[:, :], zt[:, :], -1.0, 1.0,
                                            OP.mult, OP.add)
                    nc.vector.tensor_mul(cc[:, :], zt[:, :], h[:, :])
                    nc.vector.tensor_mul(ee[:, :], nt[:, :], dd[:, :])
                    nc.vector.tensor_add(h[:, :], ee[:, :], cc[:, :])
                    # acc += h, recomputed as e + c on gpsimd so the h tile has
                    # no cross-engine reader (avoids a WAR stall on the update).
                    gp = wpool.tile([P, W4], BF, tag="gp")
                    nc.gpsimd.tensor_add(gp[:, :], ee[:, :], cc[:, :])
                    if t == 0:
                        nc.gpsimd.tensor_copy(acc[:, :], gp[:, :])
                    else:
                        nc.gpsimd.tensor_add(acc[:, :], acc[:, :], gp[:, :])

            # ---------------- Level 3 ----------------
            xt8v = xt8.rearrange("p (k c) -> p k c", k=KT)
            gi3 = cpool.tile([P, MT * TNB], BF)
            gi_matmul_fp8dr(gi3, xt8v)
            gi3v = gi3[:].rearrange("p (m t b) -> p m t b", m=MT, t=T)
            h3 = spool.tile([P, KT * NB], BF)
            acc3 = spool.tile([P, KT * NB], FP)
            gru_level(NB, h3, acc3, gi3v, zero_h0=True)

            # ---------------- Level 3 -> 2 transition ----------------
            # x2[p,k,t,b2] = acc3[p,k,b2,t]/8 (child t of parent b2 is node
            # 8*b2+t); h2 = mean over children of h3 final.
            x2 = spool.tile([P, KT * NB], BF)
            nc.vector.tensor_scalar_mul(
                x2[:].rearrange("p (k t b) -> p k t b", k=KT, t=A),
                acc3[:].rearrange("p (k b t) -> p k t b", k=KT, b=A),
                1.0 / A,
            )
            hr2 = spool.tile([P, KT * A], FP)
            nc.vector.tensor_reduce(
                hr2[:].rearrange("p (k b) -> p k b", k=KT),
                h3[:].rearrange("p (k b j) -> p k b j", k=KT, b=A),
                axis=mybir.AxisListType.X,
                op=OP.add,
            )
            h2 = spool.tile([P, KT * A], BF)
            nc.scalar.mul(h2[:, :], hr2[:, :], 1.0 / A)

            # ---------------- Level 2 ----------------
            x2v = x2.rearrange("p (k c) -> p k c", k=KT)
            gi2 = cpool.tile([P, MT * NB], BF)
            gi_matmul_e3(gi2, x2v, NB)
            gi2v = gi2[:].rearrange("p (m t b) -> p m t b", m=MT, t=T)
            acc2 = spool.tile([P, KT * A], FP)
            gru_level(A, h2, acc2, gi2v, zero_h0=False)

            # ---------------- Level 2 -> 1 transition ----------------
            x1 = spool.tile([P, KT * A], BF)
            nc.vector.tensor_scalar_mul(x1[:, :], acc2[:, :], 1.0 / A)
            hr1 = spool.tile([P, KT], FP)
            nc.vector.tensor_reduce(
                hr1[:].rearrange("p (k o) -> p k o", k=KT, o=1),
                h2[:].rearrange("p (k o j) -> p k o j", k=KT, o=1),
                axis=mybir.AxisListType.X,
                op=OP.add,
            )
            h1 = spool.tile([P, KT], BF)
            nc.scalar.mul(h1[:, :], hr1[:, :], 1.0 / A)

            # ---------------- Level 1 ----------------
            x1v = x1.rearrange("p (k c) -> p k c", k=KT)
            gi1 = cpool.tile([P, MT * A], BF)
            gi_matmul_e3(gi1, x1v, A)
            gi1v = gi1[:].rearrange("p (m t b) -> p m t b", m=MT, t=T, b=1)
            acc1 = spool.tile([P, KT], FP)
            gru_level(1, h1, acc1, gi1v, zero_h0=False)

            out_sb = spool.tile([P, KT], FP)
            nc.vector.tensor_scalar_mul(out_sb[:, :], acc1[:, :], 1.0 / A)
            nc.sync.dma_start(out=outp[:, :], in_=out_sb[:, :])

    nc.finalize()
    return nc


def _get_nc():
    global _BUILT
    if _BUILT is None:
        _BUILT = _build_nc()
    return _BUILT


def make_inputs(leaf_ids, embed_table, W_ih, W_hh, b_ih, b_hh):
    """Host-side layout prep: gather embedding rows, pre-scale, pack the
    transposed tile formats, quantize."""
    import ml_dtypes

    E4 = ml_dtypes.float8_e4m3
    E3 = ml_dtypes.float8_e3m4
    BFnp = ml_dtypes.bfloat16

    leaf_ids = np.asarray(leaf_ids).astype(np.int64)
    emb = np.asarray(embed_table, dtype=np.float32)
    W_ih = np.asarray(W_ih, dtype=np.float32)
    W_hh = np.asarray(W_hh, dtype=np.float32)
    b_ih = np.asarray(b_ih, dtype=np.float32)
    b_hh = np.asarray(b_hh, dtype=np.float32)

    x = emb[leaf_ids]  # [64, 8, 512]
    xtm = np.ascontiguousarray(x.transpose(1, 0, 2)).reshape(TNB, D)
    xt = np.ascontiguousarray(
        xtm.T.reshape(KT, P, TNB).transpose(1, 0, 2)
    ).reshape(P, KT * TNB) * XS

    def pack_w(Wsub, scale):  # [rows, 512] -> [(m,k)-major lhsT tiles]
        WT = np.ascontiguousarray(Wsub.T) * scale  # [512, rows]
        mt = Wsub.shape[0] // P
        return np.ascontiguousarray(
            WT.reshape(KT, P, mt, P).transpose(1, 2, 0, 3)
        ).reshape(P, mt * KT * P)

    blob_a = np.concatenate([xt, pack_w(W_ih, WS / XS)], axis=1).astype(E4)
    blob_b = np.concatenate([pack_w(W_hh, WS), pack_w(W_ih, WS)], axis=1).astype(BFnp)

    blob_c = np.zeros((P, C_COLS), dtype=np.float32)
    blob_c[:, OC_ID:OC_ID + P] = np.eye(P, dtype=np.float32)
    # bhn row (partition 0): col OC_BHN + g = 64*b_hh_n[g], g in [0, 512)
    blob_c[0, OC_BHN:OC_BHN + D] = WS * b_hh[2 * D:]
    blob_c = blob_c.astype(BFnp)

    gbias = WS * np.concatenate([(b_ih + b_hh)[:2 * D], b_ih[2 * D:]])
    gb_in = np.ascontiguousarray(gbias.reshape(MT, P).T)          # [P, 12]
    bhn_in = np.ascontiguousarray((WS * b_hh[2 * D:]).reshape(KT, P).T)
    bhnb_in = np.ascontiguousarray(np.repeat(bhn_in, NB, axis=1))  # [P, 256]
    blob_d = np.concatenate([gb_in, bhnb_in], axis=1).astype(np.float32)

    assert blob_a.shape == (P, A_COLS)
    assert blob_b.shape == (P, B_COLS)
    assert blob_c.shape == (P, C_COLS)
    assert blob_d.shape == (P, D_COLS)
    return {
        "blob_a": np.ascontiguousarray(blob_a),
        "blob_b": np.ascontiguousarray(blob_b),
        "blob_c": np.ascontiguousarray(blob_c),
        "blob_d": np.ascontiguousarray(blob_d),
    }


def unpack_output(out_np):
    # out [P, KT]: element (p, k) = root dim k*128+p
    return np.ascontiguousarray(out_np.T).reshape(1, 1, D).astype(np.float32)


def kernel(leaf_ids=None, layer=None, embed_table=None, W_ih=None, W_hh=None,
           b_ih=None, b_hh=None, **_unused):
    in_map = make_inputs(leaf_ids, embed_table, W_ih, W_hh, b_ih, b_hh)
    nc = _get_nc()
    res = run_bass_kernel_spmd(nc, [in_map] * N_CORES, list(range(N_CORES)))
    return unpack_output(res.results[0]["out"])


# revision 17
# speedup vs baseline: 1.2944x; 1.2944x over previous
"""Trainium2 Bass kernel for the CGF tree-GRU problem (v2).

Problem: 3-level complete 8-ary tree GRU (torch GRU cell convention).
  Level 3: 64 nodes x 8 embedded leaf children, h0 = 0
  Level 2:  8 nodes x 8 children (level-3 outputs), h0 = mean of children h
  Level 1:  1 node  x 8 children (level-2 outputs), h0 = mean of children h
  Output: mean over the 8 step outputs of the root GRU. D = 512.

One serial chain of 24 GRU steps, replicated SPMD on all 8 cores (a
per-step collective costs more than a step: AllGather floor ~4.6us).

v2 changes vs the DoubleRow baseline (154us):
  - Recurrent matmuls: fp8e3 (e3m4, 4 mantissa bits) weights WITHOUT
    DoubleRow -> Fast Weight Load.  DR at free-dim<=64 costs ~127ns/pair
    vs ~40-53ns for FWL pairs.  rhs h stays bf16 (mixed-dtype matmul).
  - Single bf16 h state (no fp8 state copy), PSUM domain = 64x true.
  - Per-step gate-input adds eliminated: gi is injected into PSUM by
    identity-weight matmuls inside each burst; sigma reads PSUM directly.
  - n-gate hidden bias injected via a ones-row matmul (k=1 tile).
  - Tail shortened to w = n - h; ft = z*w; h' = n - ft (3 DVE bf16 ops).
  - ACT order per step: sigmoid(r) -> tanh -> sigmoid(z).
  - PE warmed with dummy matmuls during the input DMA so HAM is at
    2.4GHz when the real work starts.
"""

import numpy as np

import concourse.bacc as bacc
import concourse.mybir as mybir
from concourse.tile import TileContext
from concourse.bass_utils import run_bass_kernel_spmd

AF = mybir.ActivationFunctionType
OP = mybir.AluOpType
PM = mybir.MatmulPerfMode.DoubleRow
FP = mybir.dt.float32
BF = mybir.dt.bfloat16
F8E4 = mybir.dt.float8e4
F8E3 = mybir.dt.float8e3

P = 128          # partitions
D = 512          # hidden size
KT = D // P      # 4 k-tiles
MT = 12          # gate m-tiles (3*512/128)
A = 8            # tree arity == sequence length per level
NB = 64          # level-3 node count
T = 8            # steps per level
N_CORES = 8
WS = 64.0        # weight pre-scale -> PSUM domain is 64x true values
XS = 16.0        # level-3 embedding pre-scale (fp8e4 subnormal lift)
TNB = T * NB     # 512 level-3 sequence columns
N_WARM = 22      # PE warm-up matmuls during input DMA

# blob_a (fp8e4): [xt (KT*TNB) | wit8 (MT*KT*P)]
OA_XT = 0
OA_WIT8 = OA_XT + KT * TNB
A_COLS = OA_WIT8 + MT * KT * P
# blob_b (bf16): [wrec (MT*KT*P) | wit3 (MT*KT*P)]
OB_WREC = 0
OB_WIT3 = OB_WREC + MT * KT * P
B_COLS = OB_WIT3 + MT * KT * P
# blob_c (bf16): [identity (P) | bhn_row (D), partition 0 only]
OC_ID = 0
OC_BHN = OC_ID + P
C_COLS = OC_BHN + D
# blob_d (fp32): [gb64 (MT) | bhnb64 (KT*NB)]
OD_GB = 0
OD_BHNB = OD_GB + MT
D_COLS = OD_BHNB + KT * NB

_BUILT = None


def _build_nc():
    nc = bacc.Bacc()

    blob_a = nc.declare_dram_parameter("blob_a", [P, A_COLS], F8E4, isOutput=False)
    blob_b = nc.declare_dram_parameter("blob_b", [P, B_COLS], BF, isOutput=False)
    blob_c = nc.declare_dram_parameter("blob_c", [P, C_COLS], BF, isOutput=False)
    blob_d = nc.declare_dram_parameter("blob_d", [P, D_COLS], FP, isOutput=False)
    outp = nc.declare_dram_parameter("out", [P, KT], FP, isOutput=True)

    with TileContext(nc) as tc:
        with (
            tc.tile_pool(name="const", bufs=1) as cpool,
            tc.tile_pool(name="state", bufs=1) as spool,
            tc.tile_pool(name="work", bufs=2) as wpool,
            tc.tile_pool(name="pg", bufs=2, space="PSUM") as gpool,
            tc.tile_pool(name="pw", bufs=1, space="PSUM") as wmpool,
            tc.tile_pool(name="pr", bufs=1, space="PSUM") as prpool,
            tc.tile_pool(name="pz", bufs=1, space="PSUM") as pzpool,
            tc.tile_pool(name="pn", bufs=1, space="PSUM") as pnpool,
        ):
            # ---------------- input DMA (consumption order) ----------------
            # Two queues (sync + scalar), large chunks, issued before the ACT
            # table loads so transfers start immediately.
            c_sb = cpool.tile([P, C_COLS], BF)
            nc.scalar.dma_start(out=c_sb[:], in_=blob_c[:, :])
            d_sb = cpool.tile([P, D_COLS], FP)
            nc.scalar.dma_start(out=d_sb[:], in_=blob_d[:, :])
            a_sb = cpool.tile([P, A_COLS], F8E4)
            for c0 in range(0, A_COLS, 2048):
                c1 = min(c0 + 2048, A_COLS)
                nc.sync.dma_start(out=a_sb[:, c0:c1], in_=blob_a[:, c0:c1])
            b_sb = cpool.tile([P, B_COLS], BF)
            for c0 in range(0, B_COLS, 3072):
                c1 = min(c0 + 3072, B_COLS)
                nc.sync.dma_start(out=b_sb[:, c0:c1], in_=blob_b[:, c0:c1])

            # ---------------- warm-ups ----------------
            # ACT spline tables: Sigmoid first so one ACT_TABLE_LOAD covers
            # Sigmoid/Tanh/Identity (the sigmoid set contains all three).
            wact = cpool.tile([P, 8], FP)
            nc.vector.memset(wact[:, :], 0.0)
            for fn in (AF.Sigmoid, AF.Tanh, AF.Identity):
                nc.scalar.activation(wact[:, :], wact[:, :], fn)
            # PE HAM: dummy matmuls on a zero tile while the DMA runs so the
            # clock gate is at 8/8 when the real matmuls arrive.
            warm = cpool.tile([P, P], BF)
            nc.vector.memset(warm[:, :], 0.0)
            warm_ps = wmpool.tile([P, P], FP)
            for _ in range(N_WARM):
                nc.tensor.matmul(warm_ps[:, :], lhsT=warm[:, :], rhs=warm[:, :],
                                 start=True, stop=True)

            ident = c_sb[:, OC_ID:OC_ID + P]
            gb_sb = d_sb[:, OD_GB:OD_GB + MT]
            bhnb = d_sb[:, OD_BHNB:OD_BHNB + KT * NB]
            bhnbv = bhnb.rearrange("p (k b) -> p k b", k=KT)
            xt8 = a_sb[:, OA_XT:OA_XT + KT * TNB]
            wit8 = a_sb[:, OA_WIT8:OA_WIT8 + MT * KT * P]
            wrec = b_sb[:, OB_WREC:OB_WREC + MT * KT * P]
            wit3 = b_sb[:, OB_WIT3:OB_WIT3 + MT * KT * P]

            def w8_tile(m, kk):
                # [P, 2, 128] DoubleRow stationary pair (k-tiles 2kk, 2kk+1)
                return wit8[
                    :, (m * KT + 2 * kk) * P:(m * KT + 2 * kk + 2) * P
                ].rearrange("p (two f) -> p two f", two=2)

            def wr_tile(m, k):
                return wrec[:, (m * KT + k) * P:(m * KT + k + 1) * P]

            def wi3_tile(m, k):
                return wit3[:, (m * KT + k) * P:(m * KT + k + 1) * P]

            def bhn_row(m):
                return c_sb[0:1, OC_BHN + m * P:OC_BHN + (m + 1) * P]

            # bf16 broadcast copy of the (64x) n-gate hidden bias, used as
            # the moving operand of the ps_n identity-inject matmul.
            bhnb_bf = cpool.tile([P, KT * NB], BF)
            nc.vector.tensor_copy(bhnb_bf[:, :], bhnb[:, :])
            bhnb_bfv = bhnb_bf.rearrange("p (k b) -> p k b", k=KT)

            def gi_matmul_fp8dr(gi, x_v):
                """Level-3 input matmul: fp8e4 DoubleRow, 256-col chunks."""
                for m in range(MT):
                    psb = gpool.tile([P, 512], FP, tag="gi", name="psb")
                    ps = psb[:, :TNB]
                    for c in range(2):
                        sl = ps[:, c * 256:(c + 1) * 256]
                        for kk in range(2):
                            nc.tensor.matmul(
                                sl,
                                lhsT=w8_tile(m, kk),
                                rhs=x_v[:, 2 * kk:2 * kk + 2, c * 256:(c + 1) * 256],
                                start=(kk == 0),
                                stop=(kk == 1),
                                perf_mode=PM,
                            )
                    dst = gi[:, m * TNB:(m + 1) * TNB]
                    if m % 2 == 0:
                        nc.vector.tensor_scalar_add(dst, ps[:, :], gb_sb[:, m:m + 1])
                    else:
                        nc.scalar.activation(dst, ps[:, :], AF.Identity,
                                             bias=gb_sb[:, m:m + 1], scale=1.0)

            def gi_matmul_e3(gi, x_v, cols):
                """Level-2/1 input matmul: bf16 weights x bf16 x (FWL)."""
                for m in range(MT):
                    psb = gpool.tile([P, 512], FP, tag="gi", name="psb")
                    ps = psb[:, :cols]
                    for k in range(KT):
                        nc.tensor.matmul(
                            ps,
                            lhsT=wi3_tile(m, k),
                            rhs=x_v[:, k],
                            start=(k == 0),
                            stop=(k == KT - 1),
                        )
                    dst = gi[:, m * cols:(m + 1) * cols]
                    if m % 2 == 0:
                        nc.vector.tensor_scalar_add(dst, ps[:, :], gb_sb[:, m:m + 1])
                    else:
                        nc.scalar.activation(dst, ps[:, :], AF.Identity,
                                             bias=gb_sb[:, m:m + 1], scale=1.0)

            def gru_level(B, h, acc, gi_v, zero_h0):
                """8 GRU steps.  h: [P, KT*B] bf16 (true scale); acc: [P, KT*B]
                fp32 output-mean accumulator; gi_v: [p, m, t, b] bf16 AP of the
                64x-domain biased input gates.

                Injects for step t+1 (gi via identity weights, bhn via a
                ones-row) are emitted at the end of step t so they run in the
                PE stream while step t's gate chain executes; the recurrent
                matmuls of t+1 then only wait on h."""
                W4 = 4 * B

                def v4(ap):
                    return ap.rearrange("p (m b) -> p m b", m=4)

                def emit_injects(t):
                    ps_r = prpool.tile([P, 512], FP, tag="ps_r", name="ps_r")
                    ps_z = pzpool.tile([P, 512], FP, tag="ps_z", name="ps_z")
                    ps_n = pnpool.tile([P, 512], FP, tag="ps_n", name="ps_n")
                    nc.tensor.matmul(v4(ps_r[:, :W4]), lhsT=ident,
                                     rhs=gi_v[:, 0:4, t], start=True, stop=False,
                                     skip_group_check=True)
                    nc.tensor.matmul(v4(ps_n[:, :W4]), lhsT=ident,
                                     rhs=bhnb_bfv[:, :, :B], start=True, stop=False,
                                     skip_group_check=True)
                    nc.tensor.matmul(v4(ps_z[:, :W4]), lhsT=ident,
                                     rhs=gi_v[:, 4:8, t], start=True, stop=False,
                                     skip_group_check=True)
                    return ps_r, ps_z, ps_n

                ps = None
                for t in range(T):
                    rt = wpool.tile([P, W4], BF, tag="rt")
                    zt = wpool.tile([P, W4], BF, tag="zt")
                    ctm = wpool.tile([P, W4], BF, tag="ctm")
                    ct = wpool.tile([P, W4], BF, tag="ct")
                    nt = wpool.tile([P, W4], BF, tag="nt")
                    dd = wpool.tile([P, W4], BF, tag="dd")
                    cc = wpool.tile([P, W4], BF, tag="cc")
                    ee = wpool.tile([P, W4], BF, tag="ee")

                    if t == 0 and zero_h0:
                        # h = 0: gates come straight from gi; h' = (1-z)*n
                        nc.scalar.activation(v4(rt[:, :]), gi_v[:, 0:4, 0],
                                             AF.Sigmoid, scale=1.0 / WS)
                        nc.scalar.activation(v4(zt[:, :]), gi_v[:, 4:8, 0],
                                             AF.Sigmoid, scale=1.0 / WS)
                        nc.vector.tensor_mul(v4(ctm[:, :]), v4(rt[:, :]),
                                             bhnbv[:, :, :B])
                        nc.vector.tensor_add(v4(ct[:, :]), v4(ctm[:, :]),
                                             gi_v[:, 8:12, 0])
                        nc.scalar.activation(nt[:, :], ct[:, :], AF.Tanh,
                                             scale=1.0 / WS)
                        nc.vector.tensor_scalar(dd[:, :], zt[:, :], -1.0, 1.0,
                                                OP.mult, OP.add)
                        nc.vector.tensor_mul(h[:, :], nt[:, :], dd[:, :])
                        ps = emit_injects(1)
                        nc.vector.tensor_copy(acc[:, :], h[:, :])
                        continue

                    if ps is None:  # first step of a non-zero-h0 level
                        ps = emit_injects(t)
                    ps_r, ps_z, ps_n = ps

                    # recurrent bursts: r, n, z (gi/bhn already in PSUM)
                    for m in range(4):
                        sl = ps_r[:, m * B:(m + 1) * B]
                        for k in range(KT):
                            nc.tensor.matmul(sl, lhsT=wr_tile(m, k),
                                             rhs=h[:, k * B:(k + 1) * B],
                                             start=False,
                                             stop=(m == 3 and k == KT - 1),
                                             skip_group_check=True)
                    for m in range(4):
                        sl = ps_n[:, m * B:(m + 1) * B]
                        for k in range(KT):
                            nc.tensor.matmul(sl, lhsT=wr_tile(8 + m, k),
                                             rhs=h[:, k * B:(k + 1) * B],
                                             start=False,
                                             stop=(m == 3 and k == KT - 1),
                                             skip_group_check=True)
                    for m in range(4):
                        sl = ps_z[:, m * B:(m + 1) * B]
                        for k in range(KT):
                            nc.tensor.matmul(sl, lhsT=wr_tile(4 + m, k),
                                             rhs=h[:, k * B:(k + 1) * B],
                                             start=False,
                                             stop=(m == 3 and k == KT - 1),
                                             skip_group_check=True)

                    nc.scalar.activation(rt[:, :], ps_r[:, :W4], AF.Sigmoid,
                                         scale=1.0 / WS)
                    nc.scalar.activation(zt[:, :], ps_z[:, :W4], AF.Sigmoid,
                                         scale=1.0 / WS)

                    # ct path: ctm = rt * (ps_n + bhn); ct = ctm + gi_n
                    nc.vector.tensor_mul(v4(ctm[:, :]), v4(rt[:, :]),
                                         v4(ps_n[:, :W4]))
                    nc.vector.tensor_add(v4(ct[:, :]), v4(ctm[:, :]),
                                         gi_v[:, 8:12, t])
                    nc.scalar.activation(nt[:, :], ct[:, :], AF.Tanh,
                                         scale=1.0 / WS)

                    # tail: d = 1-z ; c = z*h ; e = n*d ; h' = e + c
                    nc.vector.tensor_scalar(dd[:, :], zt[:, :], -1.0, 1.0,
                                            OP.mult, OP.add)
                    nc.vector.tensor_mul(cc[:, :], zt[:, :], h[:, :])
                    nc.vector.tensor_mul(ee[:, :], nt[:, :], dd[:, :])
                    nc.vector.tensor_add(h[:, :], ee[:, :], cc[:, :])

                    # next step's injects ride the PE stream here; emitted
                    # AFTER the h update so its WAR wait covers only the
                    # already-finished recurrent matmuls.
                    ps = emit_injects(t + 1) if t + 1 < T else None

                    # acc += h on DVE: same-queue ordering right after the h
                    # write, so the next step's tail never stalls on it.
                    if t == 0:
                        nc.vector.tensor_copy(acc[:, :], h[:, :])
                    else:
                        nc.vector.tensor_add(acc[:, :], acc[:, :], h[:, :])

            # ---------------- Level 3 ----------------
            xt8v = xt8.rearrange("p (k c) -> p k c", k=KT)
            gi3 = cpool.tile([P, MT * TNB], BF)
            gi_matmul_fp8dr(gi3, xt8v)
            gi3v = gi3[:].rearrange("p (m t b) -> p m t b", m=MT, t=T)
            h3 = spool.tile([P, KT * NB], BF)
            acc3 = spool.tile([P, KT * NB], FP)
            gru_level(NB, h3, acc3, gi3v, zero_h0=True)

            # ---------------- Level 3 -> 2 transition ----------------
            # x2[p,k,t,b2] = acc3[p,k,b2,t]/8 (child t of parent b2 is node
            # 8*b2+t); h2 = mean over children of h3 final.
            x2 = spool.tile([P, KT * NB], BF)
            nc.vector.tensor_scalar_mul(
                x2[:].rearrange("p (k t b) -> p k t b", k=KT, t=A),
                acc3[:].rearrange("p (k b t) -> p k t b", k=KT, b=A),
                1.0 / A,
            )
            hr2 = spool.tile([P, KT * A], FP)
            nc.vector.tensor_reduce(
                hr2[:].rearrange("p (k b) -> p k b", k=KT),
                h3[:].rearrange("p (k b j) -> p k b j", k=KT, b=A),
                axis=mybir.AxisListType.X,
                op=OP.add,
            )
            h2 = spool.tile([P, KT * A], BF)
            nc.scalar.mul(h2[:, :], hr2[:, :], 1.0 / A)

            # ---------------- Level 2 ----------------
            x2v = x2.rearrange("p (k c) -> p k c", k=KT)
            gi2 = cpool.tile([P, MT * NB], BF)
            gi_matmul_e3(gi2, x2v, NB)
            gi2v = gi2[:].rearrange("p (m t b) -> p m t b", m=MT, t=T)
            acc2 = spool.tile([P, KT * A], FP)
            gru_level(A, h2, acc2, gi2v, zero_h0=False)

            # ---------------- Level 2 -> 1 transition ----------------
            x1 = spool.tile([P, KT * A], BF)
            nc.vector.tensor_scalar_mul(x1[:, :], acc2[:, :], 1.0 / A)
            hr1 = spool.tile([P, KT], FP)
            nc.vector.tensor_reduce(
                hr1[:].rearrange("p (k o) -> p k o", k=KT, o=1),
                h2[:].rearrange("p (k o j) -> p k o j", k=KT, o=1),
                axis=mybir.AxisListType.X,
                op=OP.add,
            )
            h1 = spool.tile([P, KT], BF)
            nc.scalar.mul(h1[:, :], hr1[:, :], 1.0 / A)

            # ---------------- Level 1 ----------------
            x1v = x1.rearrange("p (k c) -> p k c", k=KT)
            gi1 = cpool.tile([P, MT * A], BF)
            gi_matmul_e3(gi1, x1v, A)
            gi1v = gi1[:].rearrange("p (m t b) -> p m t b", m=MT, t=T, b=1)
            acc1 = spool.tile([P, KT], FP)
            gru_level(1, h1, acc1, gi1v, zero_h0=False)

            out_sb = spool.tile([P, KT], FP)
            nc.vector.tensor_scalar_mul(out_sb[:, :], acc1[:, :], 1.0 / A)
            nc.sync.dma_start(out=outp[:, :], in_=out_sb[:, :])

    nc.finalize()
    return nc


def _get_nc():
    global _BUILT
    if _BUILT is None:
        _BUILT = _build_nc()
    return _BUILT


def make_inputs(leaf_ids, embed_table, W_ih, W_hh, b_ih, b_hh):
    """Host-side layout prep: gather embedding rows, pre-scale, pack the
    transposed tile formats, quantize."""
    import ml_dtypes

    E4 = ml_dtypes.float8_e4m3
    E3 = ml_dtypes.float8_e3m4
    BFnp = ml_dtypes.bfloat16

    leaf_ids = np.asarray(leaf_ids).astype(np.int64)
    emb = np.asarray(embed_table, dtype=np.float32)
    W_ih = np.asarray(W_ih, dtype=np.float32)
    W_hh = np.asarray(W_hh, dtype=np.float32)
    b_ih = np.asarray(b_ih, dtype=np.float32)
    b_hh = np.asarray(b_hh, dtype=np.float32)

    x = emb[leaf_ids]  # [64, 8, 512]
    xtm = np.ascontiguousarray(x.transpose(1, 0, 2)).reshape(TNB, D)
    xt = np.ascontiguousarray(
        xtm.T.reshape(KT, P, TNB).transpose(1, 0, 2)
    ).reshape(P, KT * TNB) * XS

    def pack_w(Wsub, scale):  # [rows, 512] -> [(m,k)-major lhsT tiles]
        WT = np.ascontiguousarray(Wsub.T) * scale  # [512, rows]
        mt = Wsub.shape[0] // P
        return np.ascontiguousarray(
            WT.reshape(KT, P, mt, P).transpose(1, 2, 0, 3)
        ).reshape(P, mt * KT * P)

    blob_a = np.concatenate([xt, pack_w(W_ih, WS / XS)], axis=1).astype(E4)
    blob_b = np.concatenate([pack_w(W_hh, WS), pack_w(W_ih, WS)], axis=1).astype(BFnp)

    blob_c = np.zeros((P, C_COLS), dtype=np.float32)
    blob_c[:, OC_ID:OC_ID + P] = np.eye(P, dtype=np.float32)
    # bhn row (partition 0): col OC_BHN + g = 64*b_hh_n[g], g in [0, 512)
    blob_c[0, OC_BHN:OC_BHN + D] = WS * b_hh[2 * D:]
    blob_c = blob_c.astype(BFnp)

    gbias = WS * np.concatenate([(b_ih + b_hh)[:2 * D], b_ih[2 * D:]])
    gb_in = np.ascontiguousarray(gbias.reshape(MT, P).T)          # [P, 12]
    bhn_in = np.ascontiguousarray((WS * b_hh[2 * D:]).reshape(KT, P).T)
    bhnb_in = np.ascontiguousarray(np.repeat(bhn_in, NB, axis=1))  # [P, 256]
    blob_d = np.concatenate([gb_in, bhnb_in], axis=1).astype(np.float32)

    assert blob_a.shape == (P, A_COLS)
    assert blob_b.shape == (P, B_COLS)
    assert blob_c.shape == (P, C_COLS)
    assert blob_d.shape == (P, D_COLS)
    return {
        "blob_a": np.ascontiguousarray(blob_a),
        "blob_b": np.ascontiguousarray(blob_b),
        "blob_c": np.ascontiguousarray(blob_c),
        "blob_d": np.ascontiguousarray(blob_d),
    }


def unpack_output(out_np):
    # out [P, KT]: element (p, k) = root dim k*128+p
    return np.ascontiguousarray(out_np.T).reshape(1, 1, D).astype(np.float32)


def kernel(leaf_ids=None, layer=None, embed_table=None, W_ih=None, W_hh=None,
           b_ih=None, b_hh=None, **_unused):
    in_map = make_inputs(leaf_ids, embed_table, W_ih, W_hh, b_ih, b_hh)
    nc = _get_nc()
    res = run_bass_kernel_spmd(nc, [in_map] * N_CORES, list(range(N_CORES)))
    return unpack_output(res.results[0]["out"])
